# revision 7
# baseline (speedup 1.0000x reference)
"""MoE (top-2 of 8 experts, SwiGLU MLP) Trainium2 kernel.

Strategy: expert parallelism across 8 NeuronCores. The (tiny) router runs
on host; tokens are gathered per expert on host and each core runs one
expert's SwiGLU MLP over its tokens with weights resident in SBUF.
Host applies the renormalized top-2 combine weights and scatter-adds the
two expert outputs per token.

Device layout: activations are kept transposed ([feature, token]) so every
matmul has its contraction dim on partitions with the weight tile
stationary; no on-device transposes are needed anywhere.

Head-latency optimizations vs the earlier version:
- All DRAM inputs are host-packed so every DMA is ONE contiguous 2D
  descriptor (descriptor issue on the Sync engine costs ~650 ns each, so
  the old 24-descriptor critical prefix alone was ~15 us).
- Weights are packed in per-f-block (gate/up) / per-dm-block (down) order
  so the first matmul chain only waits for a 256 KB block, not a 2 MB half.
- wu blocks are issued from the Scalar engine (HWDGE) in parallel with
  Sync's x/wg/wd stream.
- Warm-up matmuls on a zeroed tile run during the DMA wait so the PE HAM
  clock-gate is at 8/8 when the real matmuls start.
- Token cap aligned to 8 (not 32); last tile is 128 wide so the
  end-of-kernel drain (last chain + copy + DMA) is short.
- Outputs are fp16 (halves the output DMA; host combines in fp32).
"""

import sys

import numpy as np

for _p in ("/root/.axon_site", "/root/.axon_site/_ro/trn_rl_repo",
           "/root/.axon_site/_ro/pypackages", "/opt/trn_rl_repo", "/opt/pypackages"):
    if _p not in sys.path:
        sys.path.append(_p)

import ml_dtypes  # noqa: E402

import concourse.bass as bass  # noqa: E402
import concourse.tile as tile  # noqa: E402
from concourse import bacc, mybir  # noqa: E402
from concourse.bass_utils import run_bass_kernel_spmd  # noqa: E402

B, S, D, F, E, K = 4, 4096, 1024, 2048, 8, 2
N_CORES = 8
TT = 512            # token tile (PSUM bank = 512 fp32)
TOK_ALIGN = 8
ND = D // 128       # 8 d-chunks
NF = F // 128       # 16 f-chunks
ACT_DT = mybir.dt.float16
ACT_NP = np.float16
F32 = mybir.dt.float32
N_WARM = 40         # warm-up matmuls (128-wide) to flip the HAM clock gate


def _widths(tcap: int) -> list[int]:
    """Token-tile widths.

    First tiles are narrow (128, 384) so the critical DMA prefetch before
    the first matmul is small; the last tile is 128 for a short drain tail.
    """
    if tcap <= 1024:
        nfull, rem = divmod(tcap, TT)
        return [TT] * nfull + ([rem] if rem else [])
    body = tcap - 128 - 384 - 128          # after [128, 384] head, 128 tail
    nfull, rem = divmod(body, TT)
    if rem == 0:
        mids = [TT] * nfull
    elif rem >= 128:
        mids = [TT] * nfull + [rem]
    else:
        mids = [TT] * (nfull - 1) + [TT + rem - 256, 256]
    return [128, 384] + mids + [128]


def _tiles(tcap: int) -> list[tuple[int, int]]:
    tiles, o = [], 0
    for w in _widths(tcap):
        tiles.append((o, w))
        o += w
    return tiles


def _build_nc(tcap: int) -> bass.Bass:
    tiles = _tiles(tcap)

    nc = bacc.Bacc("TRN2", debug=False, target_bir_lowering=False,
                   num_devices=N_CORES)
    # x / y packed per tile: block j is [ND, w_j] (c-major, tokens contiguous)
    xt = nc.dram_tensor("xt", [128, ND * tcap], ACT_DT, kind="ExternalInput").ap()
    # weights packed per block: wg/wu block f = [ND, 128] (c-major), block
    # stride ND*128; wd block dm = [NF, 128] (f-major), block stride NF*128
    wg = nc.dram_tensor("wg", [128, NF * ND * 128], ACT_DT, kind="ExternalInput").ap()
    wu = nc.dram_tensor("wu", [128, NF * ND * 128], ACT_DT, kind="ExternalInput").ap()
    wd = nc.dram_tensor("wd", [128, ND * NF * 128], ACT_DT, kind="ExternalInput").ap()
    yt = nc.dram_tensor("yt", [128, ND * tcap], ACT_DT, kind="ExternalOutput").ap()

    WB = ND * 128     # 1024: gate/up per-f block width
    DB = NF * 128     # 2048: down per-dm block width

    with tile.TileContext(nc) as tc:
        with tc.tile_pool(name="wpool", bufs=1) as wpool, \
             tc.tile_pool(name="xpool", bufs=3) as xpool, \
             tc.tile_pool(name="hpool", bufs=2) as hpool, \
             tc.tile_pool(name="spool", bufs=3) as spool, \
             tc.tile_pool(name="opool", bufs=2) as opool, \
             tc.tile_pool(name="gp", bufs=2, space="PSUM") as gp, \
             tc.tile_pool(name="up", bufs=2, space="PSUM") as up, \
             tc.tile_pool(name="yp", bufs=2, space="PSUM") as yp, \
             tc.tile_pool(name="wmp", bufs=1, space="PSUM") as wmp:

            # --- PE warm-up on a zeroed tile (no DMA dependency) ---
            warm_sb = wpool.tile([128, 256], ACT_DT, name="warm_sb")
            nc.vector.memset(warm_sb[:], 0.0)
            warm_ps = wmp.tile([128, 128], F32, name="warm_ps")
            for _ in range(N_WARM):
                nc.tensor.matmul(warm_ps[:], warm_sb[:, 0:128],
                                 warm_sb[:, 128:256], start=True, stop=True)

            # --- resident weights, one contiguous DMA per block ---
            wg_sb = wpool.tile([128, NF * WB], ACT_DT, name="wg_sb")
            wu_sb = wpool.tile([128, NF * WB], ACT_DT, name="wu_sb")
            wd_sb = wpool.tile([128, ND * DB], ACT_DT, name="wd_sb")

            # Sync stream: x0 + wg-f0 first (the only blockers of the first
            # matmul chain), x1/x2 prefetch woven between early wg blocks,
            # then the rest of wg and wd.
            x_tiles = {}
            xbase = {}
            xb = 0
            for j, (off, w) in enumerate(tiles):
                xbase[j] = xb
                xb += ND * w
            for j in range(min(3, len(tiles))):
                x_tiles[j] = xpool.tile([128, ND * TT], ACT_DT, tag="x",
                                        name=f"x_sb{j}")
            nc.sync.dma_start(x_tiles[0][:, :ND * tiles[0][1]],
                              xt[:, xbase[0]:xbase[0] + ND * tiles[0][1]])
            for f in range(NF):
                nc.sync.dma_start(wg_sb[:, f * WB:(f + 1) * WB],
                                  wg[:, f * WB:(f + 1) * WB])
                if f == 1 and 1 in x_tiles:
                    nc.sync.dma_start(x_tiles[1][:, :ND * tiles[1][1]],
                                      xt[:, xbase[1]:xbase[1] + ND * tiles[1][1]])
                if f == 5 and 2 in x_tiles:
                    nc.sync.dma_start(x_tiles[2][:, :ND * tiles[2][1]],
                                      xt[:, xbase[2]:xbase[2] + ND * tiles[2][1]])
            for dm in range(ND):
                nc.sync.dma_start(wd_sb[:, dm * DB:(dm + 1) * DB],
                                  wd[:, dm * DB:(dm + 1) * DB])
            # up blocks ride the Scalar engine (HWDGE) in parallel; the first
            # few are issued up front, the rest interleave with tile-0 silus
            NUP = 4
            for f in range(NUP):
                nc.scalar.dma_start(wu_sb[:, f * WB:(f + 1) * WB],
                                    wu[:, f * WB:(f + 1) * WB])

            for j, (off, w) in enumerate(tiles):
                xb = xbase[j]
                x_sb = x_tiles.get(j)
                if x_sb is None:
                    x_sb = xpool.tile([128, ND * TT], ACT_DT, tag="x",
                                      name=f"x_sb{j}")
                    nc.sync.dma_start(x_sb[:, :ND * w],
                                      xt[:, xb:xb + ND * w])
                h_sb = hpool.tile([128, NF * TT], ACT_DT)
                for f in range(NF):
                    g_ps = gp.tile([128, TT], F32)
                    for c in range(ND):
                        nc.tensor.matmul(
                            g_ps[:, :w],
                            wg_sb[:, f * WB + c * 128: f * WB + (c + 1) * 128],
                            x_sb[:, c * w: (c + 1) * w],
                            start=(c == 0), stop=(c == ND - 1))
                    u_ps = up.tile([128, TT], F32)
                    for c in range(ND):
                        nc.tensor.matmul(
                            u_ps[:, :w],
                            wu_sb[:, f * WB + c * 128: f * WB + (c + 1) * 128],
                            x_sb[:, c * w: (c + 1) * w],
                            start=(c == 0), stop=(c == ND - 1))
                    s_sb = spool.tile([128, TT], F32)
                    nc.scalar.activation(s_sb[:, :w], g_ps[:, :w],
                                         mybir.ActivationFunctionType.Silu)
                    if j == 0 and NUP + f < NF:
                        fo = NUP + f
                        nc.scalar.dma_start(wu_sb[:, fo * WB:(fo + 1) * WB],
                                            wu[:, fo * WB:(fo + 1) * WB])
                    nc.vector.tensor_mul(h_sb[:, f * w: (f + 1) * w],
                                         s_sb[:, :w], u_ps[:, :w])
                o_sb = opool.tile([128, ND * TT], ACT_DT)
                for dm in range(ND):
                    y_ps = yp.tile([128, TT], F32)
                    for f in range(NF):
                        nc.tensor.matmul(
                            y_ps[:, :w],
                            wd_sb[:, dm * DB + f * 128: dm * DB + (f + 1) * 128],
                            h_sb[:, f * w: (f + 1) * w],
                            start=(f == 0), stop=(f == NF - 1))
                    nc.vector.tensor_copy(o_sb[:, dm * w: (dm + 1) * w],
                                          y_ps[:, :w])
                nc.sync.dma_start(yt[:, xb:xb + ND * w], o_sb[:, :ND * w])
    nc.compile()
    return nc


def _route(x: np.ndarray, router_w: np.ndarray):
    """Host router identical in math to the jax reference (fp32)."""
    logits = x @ router_w.T                                   # [T, E]
    logits = logits - logits.max(axis=-1, keepdims=True)
    ex = np.exp(logits, dtype=np.float32)
    scores = ex / ex.sum(axis=-1, keepdims=True)              # [T, E]
    topk_idx = np.argsort(-scores, axis=-1, kind="stable")[:, :K]   # [T, K]
    topk_w = np.take_along_axis(scores, topk_idx, axis=-1)
    topk_w = topk_w / topk_w.sum(axis=-1, keepdims=True)
    return topk_idx.astype(np.int64), topk_w.astype(np.float32)


_NC_CACHE: dict[int, bass.Bass] = {}


def _run_device(in_maps, tcap, trace=False, **kw):
    nc = _NC_CACHE.get(tcap)
    if nc is None:
        nc = _build_nc(tcap)
        _NC_CACHE[tcap] = nc
    return run_bass_kernel_spmd(nc, in_maps, core_ids=list(range(N_CORES)),
                                trace=trace, **kw)


def _prepare(hidden_states, router_w, w_gate, w_up, w_down):
    x = np.ascontiguousarray(hidden_states.reshape(-1, D)).astype(np.float32)
    topk_idx, topk_w = _route(x, router_w.astype(np.float32))

    tok_lists, w_lists = [], []
    for e in range(E):
        mask = topk_idx == e                                   # [T, K]
        tok_e = np.nonzero(mask.any(axis=1))[0]
        w_e = (topk_w * mask)[tok_e].sum(axis=1).astype(np.float32)
        tok_lists.append(tok_e)
        w_lists.append(w_e)

    max_count = max(len(t) for t in tok_lists)
    tcap = -(-max_count // TOK_ALIGN) * TOK_ALIGN
    tiles = _tiles(tcap)

    in_maps = []
    for e in range(E):
        n = len(tok_lists[e])
        xe = np.zeros((tcap, D), dtype=ACT_NP)
        xe[:n] = x[tok_lists[e]].astype(ACT_NP)
        X = xe.reshape(tcap, ND, 128).transpose(2, 1, 0)       # [128, ND, tcap]
        xt = np.concatenate(
            [np.ascontiguousarray(X[:, :, o:o + w]).reshape(128, ND * w)
             for (o, w) in tiles], axis=1)                      # [128, ND*tcap]

        wgT = np.ascontiguousarray(w_gate[e].T)                # [D, F]
        wg_d = (wgT.reshape(ND, 128, NF, 128).transpose(1, 2, 0, 3)
                .reshape(128, NF * ND * 128).astype(ACT_NP))
        wuT = np.ascontiguousarray(w_up[e].T)
        wu_d = (wuT.reshape(ND, 128, NF, 128).transpose(1, 2, 0, 3)
                .reshape(128, NF * ND * 128).astype(ACT_NP))
        wdT = np.ascontiguousarray(w_down[e].T)                # [F, D]
        wd_d = (wdT.reshape(NF, 128, ND, 128).transpose(1, 2, 0, 3)
                .reshape(128, ND * NF * 128).astype(ACT_NP))

        in_maps.append({"xt": np.ascontiguousarray(xt),
                        "wg": np.ascontiguousarray(wg_d),
                        "wu": np.ascontiguousarray(wu_d),
                        "wd": np.ascontiguousarray(wd_d)})
    return in_maps, tok_lists, w_lists, tcap


def _combine(results, tok_lists, w_lists, tcap):
    tiles = _tiles(tcap)
    out = np.zeros((B * S, D), dtype=np.float32)
    for e in range(E):
        yt = results[e]["yt"]                                  # [128, ND*tcap] f16
        n = len(tok_lists[e])
        ys = np.empty((tcap, D), dtype=np.float32)
        xb = 0
        for (o, w) in tiles:
            blk = yt[:, xb:xb + ND * w].reshape(128, ND, w)    # [p, dm, t]
            ys[o:o + w] = blk.transpose(2, 1, 0).reshape(w, D)
            xb += ND * w
        out[tok_lists[e]] += w_lists[e][:, None] * ys[:n]
    return out.reshape(B, S, D)


def kernel(hidden_states, router_w, w_gate, w_up, w_down):
    in_maps, tok_lists, w_lists, tcap = _prepare(
        hidden_states, router_w, w_gate, w_up, w_down)
    res = _run_device(in_maps, tcap)
    return _combine(res.results, tok_lists, w_lists, tcap)


def kernel_traced(hidden_states, router_w, w_gate, w_up, w_down, **kw):
    """Same as kernel() but returns (output, BassKernelResults) with NTFF trace."""
    in_maps, tok_lists, w_lists, tcap = _prepare(
        hidden_states, router_w, w_gate, w_up, w_down)
    res = _run_device(in_maps, tcap, trace=True, **kw)
    return _combine(res.results, tok_lists, w_lists, tcap), res


# revision 11
# speedup vs baseline: 1.0211x; 1.0211x over previous
"""MoE (top-2 of 8 experts, SwiGLU MLP) Trainium2 kernel.

Strategy: expert parallelism across 8 NeuronCores. The (tiny) router runs
on host; tokens are gathered per expert on host and each core runs one
expert's SwiGLU MLP over its tokens with weights resident in SBUF.
Host applies the renormalized top-2 combine weights and scatter-adds the
two expert outputs per token.

Device layout: activations are kept transposed ([feature, token]) so every
matmul has its contraction dim on partitions with the weight tile
stationary; no on-device transposes are needed anywhere.

Head-latency optimizations vs the earlier version:
- All DRAM inputs are host-packed so every DMA is ONE contiguous 2D
  descriptor (descriptor issue on the Sync engine costs ~650 ns each, so
  the old 24-descriptor critical prefix alone was ~15 us).
- Weights are packed in per-f-block (gate/up) / per-dm-block (down) order
  so the first matmul chain only waits for a 256 KB block, not a 2 MB half.
- wu blocks are issued from the Scalar engine (HWDGE) in parallel with
  Sync's x/wg/wd stream.
- Warm-up matmuls on a zeroed tile run during the DMA wait so the PE HAM
  clock-gate is at 8/8 when the real matmuls start.
- Token cap aligned to 8 (not 32); last tile is 128 wide so the
  end-of-kernel drain (last chain + copy + DMA) is short.
- Outputs are fp16 (halves the output DMA; host combines in fp32).
"""

import sys

import numpy as np

for _p in ("/root/.axon_site", "/root/.axon_site/_ro/trn_rl_repo",
           "/root/.axon_site/_ro/pypackages", "/opt/trn_rl_repo", "/opt/pypackages"):
    if _p not in sys.path:
        sys.path.append(_p)

import ml_dtypes  # noqa: E402

import concourse.bass as bass  # noqa: E402
import concourse.tile as tile  # noqa: E402
from concourse import bacc, mybir  # noqa: E402
from concourse.bass_utils import run_bass_kernel_spmd  # noqa: E402

B, S, D, F, E, K = 4, 4096, 1024, 2048, 8, 2
N_CORES = 8
TT = 512            # token tile (PSUM bank = 512 fp32)
TOK_ALIGN = 8
ND = D // 128       # 8 d-chunks
NF = F // 128       # 16 f-chunks
ACT_DT = mybir.dt.float16
ACT_NP = np.float16
F32 = mybir.dt.float32
N_WARM = 110        # warm-up matmuls (128-wide): flips the HAM clock gate and
                    # keeps the PE busy until the first weights/x land


def _widths(tcap: int) -> list[int]:
    """Token-tile widths; last tile kept at 128 for a short drain tail.

    Full-width (512) tiles everywhere else: narrower tiles consume weight
    blocks faster than HBM can deliver them during the start-up stream.
    """
    nfull, rem = divmod(tcap, TT)
    if rem == 0:
        widths = [TT] * nfull
    elif rem >= 128 or nfull == 0:
        widths = [TT] * nfull + [rem]
    else:
        widths = [TT] * (nfull - 1) + [TT + rem - 128, 128]
    return widths


def _tiles(tcap: int) -> list[tuple[int, int]]:
    tiles, o = [], 0
    for w in _widths(tcap):
        tiles.append((o, w))
        o += w
    return tiles


def _build_nc(tcap: int) -> bass.Bass:
    tiles = _tiles(tcap)

    nc = bacc.Bacc("TRN2", debug=False, target_bir_lowering=False,
                   num_devices=N_CORES)
    # x / y packed per tile: block j is [ND, w_j] (c-major, tokens contiguous)
    xt = nc.dram_tensor("xt", [128, ND * tcap], ACT_DT, kind="ExternalInput").ap()
    # weights packed per block: wg/wu block f = [ND, 128] (c-major), block
    # stride ND*128; wd block dm = [NF, 128] (f-major), block stride NF*128
    wg = nc.dram_tensor("wg", [128, NF * ND * 128], ACT_DT, kind="ExternalInput").ap()
    wu = nc.dram_tensor("wu", [128, NF * ND * 128], ACT_DT, kind="ExternalInput").ap()
    wd = nc.dram_tensor("wd", [128, ND * NF * 128], ACT_DT, kind="ExternalInput").ap()
    yt = nc.dram_tensor("yt", [128, ND * tcap], ACT_DT, kind="ExternalOutput").ap()

    WB = ND * 128     # 1024: gate/up per-f block width
    DB = NF * 128     # 2048: down per-dm block width

    with tile.TileContext(nc) as tc:
        with tc.tile_pool(name="wpool", bufs=1) as wpool, \
             tc.tile_pool(name="xpool", bufs=3) as xpool, \
             tc.tile_pool(name="hpool", bufs=2) as hpool, \
             tc.tile_pool(name="spool", bufs=3) as spool, \
             tc.tile_pool(name="opool", bufs=2) as opool, \
             tc.tile_pool(name="gp", bufs=2, space="PSUM") as gp, \
             tc.tile_pool(name="up", bufs=2, space="PSUM") as up, \
             tc.tile_pool(name="yp", bufs=2, space="PSUM") as yp, \
             tc.tile_pool(name="wmp", bufs=1, space="PSUM") as wmp:

            # --- PE warm-up on a zeroed tile (no DMA dependency) ---
            warm_sb = wpool.tile([128, 256], ACT_DT, name="warm_sb")
            nc.vector.memset(warm_sb[:], 0.0)
            warm_ps = wmp.tile([128, 128], F32, name="warm_ps")
            for _ in range(N_WARM):
                nc.tensor.matmul(warm_ps[:], warm_sb[:, 0:128],
                                 warm_sb[:, 128:256], start=True, stop=True)

            # --- resident weights, one contiguous DMA per block ---
            wg_sb = wpool.tile([128, NF * WB], ACT_DT, name="wg_sb")
            wu_sb = wpool.tile([128, NF * WB], ACT_DT, name="wu_sb")
            wd_sb = wpool.tile([128, ND * DB], ACT_DT, name="wd_sb")

            # DMA plan. Only x0 + wg-f0 + wu-f0 (the blockers of the first
            # matmul chains) may be in flight during the start-up window:
            # the SDMA engines round-robin all queued transfers at packet
            # granularity, so anything else in flight delays ALL of them.
            # - Sync: x0, wg-f0, then wd in 256 KB slices (the 3-deep DMA
            #   queue-sem pool self-paces them behind earlier completions),
            #   then x1/x2 prefetch, then per-tile y stores.
            # - Scalar: wu-f0 now; the wg/wu blocks for f>=1 are emitted with
            #   a staggered logical timestamp so the scheduler sequences them
            #   behind tile-0's silu ops - each issue then self-paces on the
            #   real progress of the matmul stream.
            x_tiles = {}
            xbase = {}
            xb = 0
            for j, (off, w) in enumerate(tiles):
                xbase[j] = xb
                xb += ND * w
            for j in range(min(3, len(tiles))):
                x_tiles[j] = xpool.tile([128, ND * TT], ACT_DT, tag="x",
                                        name=f"x_sb{j}")
            nc.sync.dma_start(x_tiles[0][:, :ND * tiles[0][1]],
                              xt[:, xbase[0]:xbase[0] + ND * tiles[0][1]])
            nc.sync.dma_start(wg_sb[:, 0:WB], wg[:, 0:WB])
            nc.scalar.dma_start(wu_sb[:, 0:WB], wu[:, 0:WB])
            for f in range(1, NF):
                with tc.tile_wait_until((1.5 + 3.4 * f) * 1e-3):
                    nc.scalar.dma_start(wg_sb[:, f * WB:(f + 1) * WB],
                                        wg[:, f * WB:(f + 1) * WB])
                    nc.scalar.dma_start(wu_sb[:, f * WB:(f + 1) * WB],
                                        wu[:, f * WB:(f + 1) * WB])
            HDB = DB // 2                       # 256 KB wd slices
            for m in range(2 * ND):
                with tc.tile_wait_until((8.0 + 1.6 * m) * 1e-3):
                    nc.sync.dma_start(wd_sb[:, m * HDB:(m + 1) * HDB],
                                      wd[:, m * HDB:(m + 1) * HDB])
            for j in (1, 2):
                if j in x_tiles:
                    with tc.tile_wait_until((30.0 + 12.0 * j) * 1e-3):
                        nc.sync.dma_start(
                            x_tiles[j][:, :ND * tiles[j][1]],
                            xt[:, xbase[j]:xbase[j] + ND * tiles[j][1]])

            for j, (off, w) in enumerate(tiles):
                xb = xbase[j]
                x_sb = x_tiles.get(j)
                if x_sb is None:
                    x_sb = xpool.tile([128, ND * TT], ACT_DT, tag="x",
                                      name=f"x_sb{j}")
                    nc.sync.dma_start(x_sb[:, :ND * w],
                                      xt[:, xb:xb + ND * w])
                h_sb = hpool.tile([128, NF * TT], ACT_DT)
                for f in range(NF):
                    g_ps = gp.tile([128, TT], F32)
                    for c in range(ND):
                        nc.tensor.matmul(
                            g_ps[:, :w],
                            wg_sb[:, f * WB + c * 128: f * WB + (c + 1) * 128],
                            x_sb[:, c * w: (c + 1) * w],
                            start=(c == 0), stop=(c == ND - 1))
                    u_ps = up.tile([128, TT], F32)
                    for c in range(ND):
                        nc.tensor.matmul(
                            u_ps[:, :w],
                            wu_sb[:, f * WB + c * 128: f * WB + (c + 1) * 128],
                            x_sb[:, c * w: (c + 1) * w],
                            start=(c == 0), stop=(c == ND - 1))
                    s_sb = spool.tile([128, TT], F32)
                    nc.scalar.activation(s_sb[:, :w], g_ps[:, :w],
                                         mybir.ActivationFunctionType.Silu)
                    nc.vector.tensor_mul(h_sb[:, f * w: (f + 1) * w],
                                         s_sb[:, :w], u_ps[:, :w])
                o_sb = opool.tile([128, ND * TT], ACT_DT)
                for dm in range(ND):
                    y_ps = yp.tile([128, TT], F32)
                    for f in range(NF):
                        nc.tensor.matmul(
                            y_ps[:, :w],
                            wd_sb[:, dm * DB + f * 128: dm * DB + (f + 1) * 128],
                            h_sb[:, f * w: (f + 1) * w],
                            start=(f == 0), stop=(f == NF - 1))
                    nc.vector.tensor_copy(o_sb[:, dm * w: (dm + 1) * w],
                                          y_ps[:, :w])
                nc.sync.dma_start(yt[:, xb:xb + ND * w], o_sb[:, :ND * w])
    nc.compile()
    return nc


def _route(x: np.ndarray, router_w: np.ndarray):
    """Host router identical in math to the jax reference (fp32)."""
    logits = x @ router_w.T                                   # [T, E]
    logits = logits - logits.max(axis=-1, keepdims=True)
    ex = np.exp(logits, dtype=np.float32)
    scores = ex / ex.sum(axis=-1, keepdims=True)              # [T, E]
    topk_idx = np.argsort(-scores, axis=-1, kind="stable")[:, :K]   # [T, K]
    topk_w = np.take_along_axis(scores, topk_idx, axis=-1)
    topk_w = topk_w / topk_w.sum(axis=-1, keepdims=True)
    return topk_idx.astype(np.int64), topk_w.astype(np.float32)


_NC_CACHE: dict[int, bass.Bass] = {}


def _run_device(in_maps, tcap, trace=False, **kw):
    nc = _NC_CACHE.get(tcap)
    if nc is None:
        nc = _build_nc(tcap)
        _NC_CACHE[tcap] = nc
    return run_bass_kernel_spmd(nc, in_maps, core_ids=list(range(N_CORES)),
                                trace=trace, **kw)


def _prepare(hidden_states, router_w, w_gate, w_up, w_down):
    x = np.ascontiguousarray(hidden_states.reshape(-1, D)).astype(np.float32)
    topk_idx, topk_w = _route(x, router_w.astype(np.float32))

    tok_lists, w_lists = [], []
    for e in range(E):
        mask = topk_idx == e                                   # [T, K]
        tok_e = np.nonzero(mask.any(axis=1))[0]
        w_e = (topk_w * mask)[tok_e].sum(axis=1).astype(np.float32)
        tok_lists.append(tok_e)
        w_lists.append(w_e)

    max_count = max(len(t) for t in tok_lists)
    tcap = -(-max_count // TOK_ALIGN) * TOK_ALIGN
    tiles = _tiles(tcap)

    in_maps = []
    for e in range(E):
        n = len(tok_lists[e])
        xe = np.zeros((tcap, D), dtype=ACT_NP)
        xe[:n] = x[tok_lists[e]].astype(ACT_NP)
        X = xe.reshape(tcap, ND, 128).transpose(2, 1, 0)       # [128, ND, tcap]
        xt = np.concatenate(
            [np.ascontiguousarray(X[:, :, o:o + w]).reshape(128, ND * w)
             for (o, w) in tiles], axis=1)                      # [128, ND*tcap]

        wgT = np.ascontiguousarray(w_gate[e].T)                # [D, F]
        wg_d = (wgT.reshape(ND, 128, NF, 128).transpose(1, 2, 0, 3)
                .reshape(128, NF * ND * 128).astype(ACT_NP))
        wuT = np.ascontiguousarray(w_up[e].T)
        wu_d = (wuT.reshape(ND, 128, NF, 128).transpose(1, 2, 0, 3)
                .reshape(128, NF * ND * 128).astype(ACT_NP))
        wdT = np.ascontiguousarray(w_down[e].T)                # [F, D]
        wd_d = (wdT.reshape(NF, 128, ND, 128).transpose(1, 2, 0, 3)
                .reshape(128, ND * NF * 128).astype(ACT_NP))

        in_maps.append({"xt": np.ascontiguousarray(xt),
                        "wg": np.ascontiguousarray(wg_d),
                        "wu": np.ascontiguousarray(wu_d),
                        "wd": np.ascontiguousarray(wd_d)})
    return in_maps, tok_lists, w_lists, tcap


def _combine(results, tok_lists, w_lists, tcap):
    tiles = _tiles(tcap)
    out = np.zeros((B * S, D), dtype=np.float32)
    for e in range(E):
        yt = results[e]["yt"]                                  # [128, ND*tcap] f16
        n = len(tok_lists[e])
        ys = np.empty((tcap, D), dtype=np.float32)
        xb = 0
        for (o, w) in tiles:
            blk = yt[:, xb:xb + ND * w].reshape(128, ND, w)    # [p, dm, t]
            ys[o:o + w] = blk.transpose(2, 1, 0).reshape(w, D)
            xb += ND * w
        out[tok_lists[e]] += w_lists[e][:, None] * ys[:n]
    return out.reshape(B, S, D)


def kernel(hidden_states, router_w, w_gate, w_up, w_down):
    in_maps, tok_lists, w_lists, tcap = _prepare(
        hidden_states, router_w, w_gate, w_up, w_down)
    res = _run_device(in_maps, tcap)
    return _combine(res.results, tok_lists, w_lists, tcap)


def kernel_traced(hidden_states, router_w, w_gate, w_up, w_down, **kw):
    """Same as kernel() but returns (output, BassKernelResults) with NTFF trace."""
    in_maps, tok_lists, w_lists, tcap = _prepare(
        hidden_states, router_w, w_gate, w_up, w_down)
    res = _run_device(in_maps, tcap, trace=True, **kw)
    return _combine(res.results, tok_lists, w_lists, tcap), res


# revision 13
# speedup vs baseline: 1.0220x; 1.0009x over previous
"""MoE (top-2 of 8 experts, SwiGLU MLP) Trainium2 kernel.

Strategy: expert parallelism across 8 NeuronCores. The (tiny) router runs
on host; tokens are gathered per expert on host and each core runs one
expert's SwiGLU MLP over its tokens with weights resident in SBUF.
Host applies the renormalized top-2 combine weights and scatter-adds the
two expert outputs per token.

Device layout: activations are kept transposed ([feature, token]) so every
matmul has its contraction dim on partitions with the weight tile
stationary; no on-device transposes are needed anywhere.

Head-latency optimizations vs the earlier version:
- All DRAM inputs are host-packed so every DMA is ONE contiguous 2D
  descriptor (descriptor issue on the Sync engine costs ~650 ns each, so
  the old 24-descriptor critical prefix alone was ~15 us).
- Weights are packed in per-f-block (gate/up) / per-dm-block (down) order
  so the first matmul chain only waits for a 256 KB block, not a 2 MB half.
- wu blocks are issued from the Scalar engine (HWDGE) in parallel with
  Sync's x/wg/wd stream.
- Warm-up matmuls on a zeroed tile run during the DMA wait so the PE HAM
  clock-gate is at 8/8 when the real matmuls start.
- Token cap aligned to 8 (not 32); last tile is 128 wide so the
  end-of-kernel drain (last chain + copy + DMA) is short.
- Outputs are fp16 (halves the output DMA; host combines in fp32).
"""

import sys

import numpy as np

for _p in ("/root/.axon_site", "/root/.axon_site/_ro/trn_rl_repo",
           "/root/.axon_site/_ro/pypackages", "/opt/trn_rl_repo", "/opt/pypackages"):
    if _p not in sys.path:
        sys.path.append(_p)

import ml_dtypes  # noqa: E402

import concourse.bass as bass  # noqa: E402
import concourse.tile as tile  # noqa: E402
from concourse import bacc, mybir  # noqa: E402
from concourse.bass_utils import run_bass_kernel_spmd  # noqa: E402

B, S, D, F, E, K = 4, 4096, 1024, 2048, 8, 2
N_CORES = 8
TT = 512            # token tile (PSUM bank = 512 fp32)
TOK_ALIGN = 8
ND = D // 128       # 8 d-chunks
NF = F // 128       # 16 f-chunks
ACT_DT = mybir.dt.float16
ACT_NP = np.float16
F32 = mybir.dt.float32
N_WARM = 90         # warm-up matmuls (128-wide): flips the HAM clock gate and
                    # keeps the PE busy until the first weights/x land


def _widths(tcap: int) -> list[int]:
    """Token-tile widths; last tile kept at 128 for a short drain tail.

    Full-width (512) tiles everywhere else: narrower tiles consume weight
    blocks faster than HBM can deliver them during the start-up stream.
    """
    nfull, rem = divmod(tcap, TT)
    if rem == 0:
        widths = [TT] * nfull
    elif rem >= 128 or nfull == 0:
        widths = [TT] * nfull + [rem]
    else:
        widths = [TT] * (nfull - 1) + [TT + rem - 128, 128]
    return widths


def _tiles(tcap: int) -> list[tuple[int, int]]:
    tiles, o = [], 0
    for w in _widths(tcap):
        tiles.append((o, w))
        o += w
    return tiles


def _build_nc(tcap: int) -> bass.Bass:
    tiles = _tiles(tcap)

    nc = bacc.Bacc("TRN2", debug=False, target_bir_lowering=False,
                   num_devices=N_CORES)
    # x / y packed per tile: block j is [ND, w_j] (c-major, tokens contiguous)
    xt = nc.dram_tensor("xt", [128, ND * tcap], ACT_DT, kind="ExternalInput").ap()
    # weights packed per block: wg/wu block f = [ND, 128] (c-major), block
    # stride ND*128; wd block dm = [NF, 128] (f-major), block stride NF*128
    wg = nc.dram_tensor("wg", [128, NF * ND * 128], ACT_DT, kind="ExternalInput").ap()
    wu = nc.dram_tensor("wu", [128, NF * ND * 128], ACT_DT, kind="ExternalInput").ap()
    wd = nc.dram_tensor("wd", [128, ND * NF * 128], ACT_DT, kind="ExternalInput").ap()
    yt = nc.dram_tensor("yt", [128, ND * tcap], ACT_DT, kind="ExternalOutput").ap()

    WB = ND * 128     # 1024: gate/up per-f block width
    DB = NF * 128     # 2048: down per-dm block width

    with tile.TileContext(nc) as tc:
        with tc.tile_pool(name="wpool", bufs=1) as wpool, \
             tc.tile_pool(name="xpool", bufs=3) as xpool, \
             tc.tile_pool(name="hpool", bufs=2) as hpool, \
             tc.tile_pool(name="spool", bufs=3) as spool, \
             tc.tile_pool(name="opool", bufs=2) as opool, \
             tc.tile_pool(name="gp", bufs=2, space="PSUM") as gp, \
             tc.tile_pool(name="up", bufs=2, space="PSUM") as up, \
             tc.tile_pool(name="yp", bufs=2, space="PSUM") as yp, \
             tc.tile_pool(name="wmp", bufs=1, space="PSUM") as wmp:

            # --- PE warm-up on a zeroed tile (no DMA dependency) ---
            warm_sb = wpool.tile([128, 256], ACT_DT, name="warm_sb")
            nc.vector.memset(warm_sb[:], 0.0)
            warm_ps = wmp.tile([128, 128], F32, name="warm_ps")
            for _ in range(N_WARM):
                nc.tensor.matmul(warm_ps[:], warm_sb[:, 0:128],
                                 warm_sb[:, 128:256], start=True, stop=True)

            # --- resident weights, one contiguous DMA per block ---
            wg_sb = wpool.tile([128, NF * WB], ACT_DT, name="wg_sb")
            wu_sb = wpool.tile([128, NF * WB], ACT_DT, name="wu_sb")
            wd_sb = wpool.tile([128, ND * DB], ACT_DT, name="wd_sb")

            # DMA plan. Only x0 + wg-f0 + wu-f0 (the blockers of the first
            # matmul chains) may be in flight during the start-up window:
            # the SDMA engines round-robin all queued transfers at packet
            # granularity, so anything else in flight delays ALL of them.
            # - Sync: x0, wg-f0, then wd in 256 KB slices (the 3-deep DMA
            #   queue-sem pool self-paces them behind earlier completions),
            #   then x1/x2 prefetch, then per-tile y stores.
            # - Scalar: wu-f0 now; the wg/wu blocks for f>=1 are emitted with
            #   a staggered logical timestamp so the scheduler sequences them
            #   behind tile-0's silu ops - each issue then self-paces on the
            #   real progress of the matmul stream.
            x_tiles = {}
            xbase = {}
            xb = 0
            for j, (off, w) in enumerate(tiles):
                xbase[j] = xb
                xb += ND * w
            for j in range(min(3, len(tiles))):
                x_tiles[j] = xpool.tile([128, ND * TT], ACT_DT, tag="x",
                                        name=f"x_sb{j}")
            nc.sync.dma_start(x_tiles[0][:, :ND * tiles[0][1]],
                              xt[:, xbase[0]:xbase[0] + ND * tiles[0][1]])
            nc.sync.dma_start(wg_sb[:, 0:WB], wg[:, 0:WB])
            nc.scalar.dma_start(wu_sb[:, 0:WB], wu[:, 0:WB])
            for f in range(1, NF):
                with tc.tile_wait_until((1.5 + 3.4 * f) * 1e-3):
                    nc.scalar.dma_start(wg_sb[:, f * WB:(f + 1) * WB],
                                        wg[:, f * WB:(f + 1) * WB])
                    nc.scalar.dma_start(wu_sb[:, f * WB:(f + 1) * WB],
                                        wu[:, f * WB:(f + 1) * WB])
            HDB = DB // 2                       # 256 KB wd slices
            for m in range(2 * ND):
                with tc.tile_wait_until((8.0 + 1.6 * m) * 1e-3):
                    nc.sync.dma_start(wd_sb[:, m * HDB:(m + 1) * HDB],
                                      wd[:, m * HDB:(m + 1) * HDB])
            for j in (1, 2):
                if j in x_tiles:
                    with tc.tile_wait_until((30.0 + 12.0 * j) * 1e-3):
                        nc.sync.dma_start(
                            x_tiles[j][:, :ND * tiles[j][1]],
                            xt[:, xbase[j]:xbase[j] + ND * tiles[j][1]])

            for j, (off, w) in enumerate(tiles):
                xb = xbase[j]
                x_sb = x_tiles.get(j)
                if x_sb is None:
                    x_sb = xpool.tile([128, ND * TT], ACT_DT, tag="x",
                                      name=f"x_sb{j}")
                    nc.sync.dma_start(x_sb[:, :ND * w],
                                      xt[:, xb:xb + ND * w])
                h_sb = hpool.tile([128, NF * TT], ACT_DT)
                for f in range(NF):
                    g_ps = gp.tile([128, TT], F32)
                    for c in range(ND):
                        nc.tensor.matmul(
                            g_ps[:, :w],
                            wg_sb[:, f * WB + c * 128: f * WB + (c + 1) * 128],
                            x_sb[:, c * w: (c + 1) * w],
                            start=(c == 0), stop=(c == ND - 1))
                    u_ps = up.tile([128, TT], F32)
                    for c in range(ND):
                        nc.tensor.matmul(
                            u_ps[:, :w],
                            wu_sb[:, f * WB + c * 128: f * WB + (c + 1) * 128],
                            x_sb[:, c * w: (c + 1) * w],
                            start=(c == 0), stop=(c == ND - 1))
                    s_sb = spool.tile([128, TT], F32)
                    nc.scalar.activation(s_sb[:, :w], g_ps[:, :w],
                                         mybir.ActivationFunctionType.Silu)
                    nc.vector.tensor_mul(h_sb[:, f * w: (f + 1) * w],
                                         s_sb[:, :w], u_ps[:, :w])
                o_sb = opool.tile([128, ND * TT], ACT_DT)
                last = j == len(tiles) - 1
                for dm in range(ND):
                    y_ps = yp.tile([128, TT], F32)
                    for f in range(NF):
                        nc.tensor.matmul(
                            y_ps[:, :w],
                            wd_sb[:, dm * DB + f * 128: dm * DB + (f + 1) * 128],
                            h_sb[:, f * w: (f + 1) * w],
                            start=(f == 0), stop=(f == NF - 1))
                    nc.vector.tensor_copy(o_sb[:, dm * w: (dm + 1) * w],
                                          y_ps[:, :w])
                    if last:
                        # per-dm stores on the final tile: the first 7 land
                        # while the last chains still run, shortening the tail
                        nc.sync.dma_start(yt[:, xb + dm * w:xb + (dm + 1) * w],
                                          o_sb[:, dm * w: (dm + 1) * w])
                if not last:
                    nc.sync.dma_start(yt[:, xb:xb + ND * w], o_sb[:, :ND * w])
    nc.compile()
    return nc


def _route(x: np.ndarray, router_w: np.ndarray):
    """Host router identical in math to the jax reference (fp32)."""
    logits = x @ router_w.T                                   # [T, E]
    logits = logits - logits.max(axis=-1, keepdims=True)
    ex = np.exp(logits, dtype=np.float32)
    scores = ex / ex.sum(axis=-1, keepdims=True)              # [T, E]
    topk_idx = np.argsort(-scores, axis=-1, kind="stable")[:, :K]   # [T, K]
    topk_w = np.take_along_axis(scores, topk_idx, axis=-1)
    topk_w = topk_w / topk_w.sum(axis=-1, keepdims=True)
    return topk_idx.astype(np.int64), topk_w.astype(np.float32)


_NC_CACHE: dict[int, bass.Bass] = {}


def _run_device(in_maps, tcap, trace=False, **kw):
    nc = _NC_CACHE.get(tcap)
    if nc is None:
        nc = _build_nc(tcap)
        _NC_CACHE[tcap] = nc
    return run_bass_kernel_spmd(nc, in_maps, core_ids=list(range(N_CORES)),
                                trace=trace, **kw)


def _prepare(hidden_states, router_w, w_gate, w_up, w_down):
    x = np.ascontiguousarray(hidden_states.reshape(-1, D)).astype(np.float32)
    topk_idx, topk_w = _route(x, router_w.astype(np.float32))

    tok_lists, w_lists = [], []
    for e in range(E):
        mask = topk_idx == e                                   # [T, K]
        tok_e = np.nonzero(mask.any(axis=1))[0]
        w_e = (topk_w * mask)[tok_e].sum(axis=1).astype(np.float32)
        tok_lists.append(tok_e)
        w_lists.append(w_e)

    max_count = max(len(t) for t in tok_lists)
    tcap = -(-max_count // TOK_ALIGN) * TOK_ALIGN
    tiles = _tiles(tcap)

    in_maps = []
    for e in range(E):
        n = len(tok_lists[e])
        xe = np.zeros((tcap, D), dtype=ACT_NP)
        xe[:n] = x[tok_lists[e]].astype(ACT_NP)
        X = xe.reshape(tcap, ND, 128).transpose(2, 1, 0)       # [128, ND, tcap]
        xt = np.concatenate(
            [np.ascontiguousarray(X[:, :, o:o + w]).reshape(128, ND * w)
             for (o, w) in tiles], axis=1)                      # [128, ND*tcap]

        wgT = np.ascontiguousarray(w_gate[e].T)                # [D, F]
        wg_d = (wgT.reshape(ND, 128, NF, 128).transpose(1, 2, 0, 3)
                .reshape(128, NF * ND * 128).astype(ACT_NP))
        wuT = np.ascontiguousarray(w_up[e].T)
        wu_d = (wuT.reshape(ND, 128, NF, 128).transpose(1, 2, 0, 3)
                .reshape(128, NF * ND * 128).astype(ACT_NP))
        wdT = np.ascontiguousarray(w_down[e].T)                # [F, D]
        wd_d = (wdT.reshape(NF, 128, ND, 128).transpose(1, 2, 0, 3)
                .reshape(128, ND * NF * 128).astype(ACT_NP))

        in_maps.append({"xt": np.ascontiguousarray(xt),
                        "wg": np.ascontiguousarray(wg_d),
                        "wu": np.ascontiguousarray(wu_d),
                        "wd": np.ascontiguousarray(wd_d)})
    return in_maps, tok_lists, w_lists, tcap


def _combine(results, tok_lists, w_lists, tcap):
    tiles = _tiles(tcap)
    out = np.zeros((B * S, D), dtype=np.float32)
    for e in range(E):
        yt = results[e]["yt"]                                  # [128, ND*tcap] f16
        n = len(tok_lists[e])
        ys = np.empty((tcap, D), dtype=np.float32)
        xb = 0
        for (o, w) in tiles:
            blk = yt[:, xb:xb + ND * w].reshape(128, ND, w)    # [p, dm, t]
            ys[o:o + w] = blk.transpose(2, 1, 0).reshape(w, D)
            xb += ND * w
        out[tok_lists[e]] += w_lists[e][:, None] * ys[:n]
    return out.reshape(B, S, D)


def kernel(hidden_states, router_w, w_gate, w_up, w_down):
    in_maps, tok_lists, w_lists, tcap = _prepare(
        hidden_states, router_w, w_gate, w_up, w_down)
    res = _run_device(in_maps, tcap)
    return _combine(res.results, tok_lists, w_lists, tcap)


def kernel_traced(hidden_states, router_w, w_gate, w_up, w_down, **kw):
    """Same as kernel() but returns (output, BassKernelResults) with NTFF trace."""
    in_maps, tok_lists, w_lists, tcap = _prepare(
        hidden_states, router_w, w_gate, w_up, w_down)
    res = _run_device(in_maps, tcap, trace=True, **kw)
    return _combine(res.results, tok_lists, w_lists, tcap), res
